# revision 2
# baseline (speedup 1.0000x reference)
"""Trainium2 Bass kernel for BinaryTimedPSP (causal boxcar window sum + clip).

psp[t] = clip(sum_{k=max(0,t-D+1)}^{t} x[k], 0, 1) over a [T=2048, B=16,
N=2048] f32 binary spike tensor, D = duration (100).

Since x is {0,1} and the result is clipped to [0,1], the output is exactly
the sliding-window OR of the spikes along time:

    out[t] = OR_{k=max(0,t-D+1)}^{t} x[k]

which is computed entirely in a 1-bit-per-element packed domain:

  - The host losslessly re-encodes each core's [T, 4096] spike slab as
    uint32 words packing 32 *spatial* columns per word -> exactly 128 words
    = the SBUF partition dim.  SBUF layout [128 words, T time] puts time
    along the free dim, so a time-shift is just a free-dim offset.
  - The windowed OR is a log-doubling ladder of DVE tensor_tensor
    bitwise_or ops with shifted operand views (bitwise ops are DVE-only,
    32-bit only on TRN2):
        g_{2w}[t] = g_w[t] | g_w[t-w]
    and, because OR is idempotent, the final window of length D is one
    extra op with overlapping coverage:
        out[t] = g_P[t] | g_P[t-(D-P)],  P = 2^floor(log2 D)
    -> ceil(log2 D) = 7 ops for D=100, the provable minimum (each op can
    at most double the covered window).
  - Each DVE op runs at 1 elem/lane/cycle on uint32 (32 bits/lane/cycle,
    the TT port ceiling), ~2.3us per full 2148-col pass.
  - A PAD=D leading zero region (host-provided) makes window truncation at
    t < D-1 automatic; uninitialized-prefix garbage in the level buffers
    provably never propagates past col D-1 < PAD.
  - Time is chunked (640/640/512/256) in a wavefront: each chunk's input
    DMA, 7 ladder ops, and output DMA pipeline against other chunks.
    Chunk halos need no duplicated work: each level writes one full-width
    buffer, so a chunk's shifted reads hit columns the previous chunk
    already produced.  All input chunks ride the scalar HWDGE ring (in
    order, so chunk 0 lands first); outputs alternate sync/scalar.
  - I/O per core is 1.07 MiB in + 1 MiB out (vs 8.4+8.4 for fp8) -- the
    kernel is DVE-bound at ~16us compute + ~13us fixed launch overheads.

Exactness: every op is an integer bitwise OR on exact bit encodings; the
result is bit-exact vs the reference. Pure data parallel over 8 cores
(4096 spatial columns each); the gather is a host-side concatenate.
"""

import numpy as np

T_FULL, B_FULL, N_FULL = 2048, 16, 2048
NCORES = 8
P = 128
COLS = B_FULL * N_FULL            # 32768 spatial columns
FREE_BITS = COLS // NCORES        # 4096 bits per core
WORDS = FREE_BITS // 32           # 128 uint32 words -> partition dim
CHUNKS = (640, 640, 512, 256)     # time chunking for DMA/compute overlap

_CACHE: dict = {}


def _shifts(d: int):
    """Ladder shifts: doubling to P = 2^floor(log2 d), then one overlap op."""
    shifts = []
    w = 1
    while 2 * w <= d:
        shifts.append(w)
        w *= 2
    if w < d:
        shifts.append(d - w)
    return shifts


def _pad(d: int) -> int:
    return ((d + 3) // 4) * 4


def _build(d: int):
    import concourse.bacc as bacc
    import concourse.mybir as mybir
    from concourse.tile import TileContext

    u32 = mybir.dt.uint32
    bor = mybir.AluOpType.bitwise_or

    PAD = _pad(d)
    END = PAD + T_FULL
    shifts = _shifts(d)
    nlev = len(shifts)
    bs = []
    acc = PAD
    for c in CHUNKS:
        acc += c
        bs.append(acc)
    assert bs[-1] == END

    nc = bacc.Bacc(None)
    x = nc.dram_tensor("x", [P, END], u32, kind="ExternalInput")
    y = nc.dram_tensor("y", [P, T_FULL], u32, kind="ExternalOutput")

    with TileContext(nc) as tc:
        with tc.tile_pool(name="pool", bufs=1) as pool:
            # one full-width buffer per ladder level: chunk halo reads are
            # just earlier columns of the same buffer, no aliasing hazards
            bufs = [
                pool.tile([P, END], u32, tag=f"L{k}", name=f"L{k}")
                for k in range(nlev + 1)
            ]
            for j, b in enumerate(bs):
                lo = 0 if j == 0 else bs[j - 1]
                # all input chunks ordered on the scalar HWDGE ring so
                # chunk 0 lands first and compute starts earliest
                nc.scalar.dma_start(out=bufs[0][:, lo:b], in_=x[:, lo:b])
                for k, s in enumerate(shifts):
                    # first chunk also fills the pad region so later
                    # chunks' halo reads below PAD see window-correct bits
                    a = s if j == 0 else max(s, bs[j - 1])
                    nc.vector.tensor_tensor(
                        out=bufs[k + 1][:, a:b],
                        in0=bufs[k][:, a:b],
                        in1=bufs[k][:, a - s:b - s],
                        op=bor,
                    )
                olo = PAD if j == 0 else bs[j - 1]
                q = nc.sync if j % 2 == 0 else nc.scalar
                q.dma_start(out=y[:, olo - PAD:b - PAD], in_=bufs[nlev][:, olo:b])
    nc.finalize()
    return nc


def _get_built(d: int):
    if d not in _CACHE:
        _CACHE[d] = _build(d)
    return _CACHE[d]


def kernel(input_spikes, duration, _trace=False):
    from concourse.bass_utils import run_bass_kernel_spmd

    x = np.asarray(input_spikes)
    d = int(duration)
    assert x.shape == (T_FULL, B_FULL, N_FULL), x.shape

    if d <= 0:
        out = np.zeros(x.shape, dtype=np.float32)
        return (out, None) if _trace else out
    if d == 1:
        # window of length 1: output is the (binary) input itself
        out = (x != 0).astype(np.float32).reshape(x.shape)
        return (out, None) if _trace else out
    d = min(d, T_FULL)  # any window >= T covers [0, t] for every t

    PAD = _pad(d)
    nc = _get_built(d)

    bits = x.reshape(T_FULL, COLS) != 0
    in_maps = []
    for core in range(NCORES):
        # pack 32 spatial columns per uint32 (bit b of word w = col 32w+b),
        # transpose to [word, time], prepend PAD zero columns
        packed = np.packbits(
            bits[:, core * FREE_BITS:(core + 1) * FREE_BITS],
            axis=1, bitorder="little",
        ).view(np.uint32)
        arr = np.zeros((P, PAD + T_FULL), dtype=np.uint32)
        arr[:, PAD:] = packed.T
        in_maps.append({"x": arr})

    res = run_bass_kernel_spmd(
        nc, in_maps, core_ids=list(range(NCORES)), trace=_trace
    )

    out = np.empty((T_FULL, COLS), dtype=np.float32)
    for core in range(NCORES):
        yv = res.results[core]["y"]  # [P, T] uint32
        ub = np.unpackbits(
            np.ascontiguousarray(yv.T).view(np.uint8), axis=1, bitorder="little"
        )  # [T, FREE_BITS] uint8 in {0,1}
        out[:, core * FREE_BITS:(core + 1) * FREE_BITS] = ub
    out = out.reshape(T_FULL, B_FULL, N_FULL)
    if _trace:
        return out, res
    return out


# revision 3
# speedup vs baseline: 1.0464x; 1.0464x over previous
"""Trainium2 Bass kernel for BinaryTimedPSP (causal boxcar window sum + clip).

psp[t] = clip(sum_{k=max(0,t-D+1)}^{t} x[k], 0, 1) over a [T=2048, B=16,
N=2048] f32 binary spike tensor, D = duration (100).

Since x is {0,1} and the result is clipped to [0,1], the output is exactly
the sliding-window OR of the spikes along time:

    out[t] = OR_{k=max(0,t-D+1)}^{t} x[k]

which is computed entirely in a 1-bit-per-element packed domain:

  - The host losslessly re-encodes each core's [T, 4096] spike slab as
    uint32 words packing 32 *spatial* columns per word -> exactly 128 words
    = the SBUF partition dim.  SBUF layout [128 words, T time] puts time
    along the free dim, so a time-shift is just a free-dim offset.
  - The windowed OR is a log-doubling ladder of DVE tensor_tensor
    bitwise_or ops with shifted operand views (bitwise ops are DVE-only,
    32-bit only on TRN2):
        g_{2w}[t] = g_w[t] | g_w[t-w]
    and, because OR is idempotent, the final window of length D is one
    extra op with overlapping coverage:
        out[t] = g_P[t] | g_P[t-(D-P)],  P = 2^floor(log2 D)
    -> ceil(log2 D) = 7 ops for D=100, the provable minimum (each op can
    at most double the covered window).
  - Each DVE op runs at 1 elem/lane/cycle on uint32 (32 bits/lane/cycle,
    the TT port ceiling), ~2.3us per full 2148-col pass.
  - A PAD=D leading zero region (host-provided) makes window truncation at
    t < D-1 automatic; uninitialized-prefix garbage in the level buffers
    provably never propagates past col D-1 < PAD.
  - Time is chunked (640/640/512/256) in a wavefront: each chunk's input
    DMA, 7 ladder ops, and output DMA pipeline against other chunks.
    Chunk halos need no duplicated work: each level writes one full-width
    buffer, so a chunk's shifted reads hit columns the previous chunk
    already produced.  All input chunks ride the scalar HWDGE ring (in
    order, so chunk 0 lands first); outputs alternate sync/scalar.
  - I/O per core is 1.07 MiB in + 1 MiB out (vs 8.4+8.4 for fp8) -- the
    kernel is DVE-bound at ~16us compute + ~13us fixed launch overheads.

Exactness: every op is an integer bitwise OR on exact bit encodings; the
result is bit-exact vs the reference. Pure data parallel over 8 cores
(4096 spatial columns each); the gather is a host-side concatenate.
"""

import numpy as np

T_FULL, B_FULL, N_FULL = 2048, 16, 2048
NCORES = 8
P = 128
COLS = B_FULL * N_FULL            # 32768 spatial columns
FREE_BITS = COLS // NCORES        # 4096 bits per core
WORDS = FREE_BITS // 32           # 128 uint32 words -> partition dim
CHUNKS = (640, 640, 512, 256)     # time chunking for DMA/compute overlap

_CACHE: dict = {}


def _shifts(d: int):
    """Ladder shifts: doubling to P = 2^floor(log2 d), then one overlap op."""
    shifts = []
    w = 1
    while 2 * w <= d:
        shifts.append(w)
        w *= 2
    if w < d:
        shifts.append(d - w)
    return shifts


def _pad(d: int) -> int:
    return ((d + 3) // 4) * 4


def _build(d: int):
    import concourse.bacc as bacc
    import concourse.mybir as mybir
    from concourse.tile import TileContext

    u32 = mybir.dt.uint32
    bor = mybir.AluOpType.bitwise_or

    PAD = _pad(d)
    END = PAD + T_FULL
    shifts = _shifts(d)
    nlev = len(shifts)
    bs = []
    acc = PAD
    for c in CHUNKS:
        acc += c
        bs.append(acc)
    assert bs[-1] == END

    nc = bacc.Bacc(None)
    x = nc.dram_tensor("x", [P, END], u32, kind="ExternalInput")
    y = nc.dram_tensor("y", [P, T_FULL], u32, kind="ExternalOutput")

    with TileContext(nc) as tc:
        with tc.tile_pool(name="pool", bufs=1) as pool:
            # one full-width buffer per ladder level: chunk halo reads are
            # just earlier columns of the same buffer, no aliasing hazards
            bufs = [
                pool.tile([P, END], u32, tag=f"L{k}", name=f"L{k}")
                for k in range(nlev + 1)
            ]
            # chunk-0 start col per level: level k is only ever read at
            # cols >= PAD - (remaining lookback) = cum_shift_k + PAD-(d-1)
            a0 = []
            cum = 0
            for s in shifts:
                cum += s
                a0.append(cum + PAD - (d - 1))
            for j, b in enumerate(bs):
                lo = 0 if j == 0 else bs[j - 1]
                # all input chunks ordered on the scalar HWDGE ring so
                # chunk 0 lands first and compute starts earliest
                nc.scalar.dma_start(out=bufs[0][:, lo:b], in_=x[:, lo:b])
                for k, s in enumerate(shifts):
                    # first chunk also fills the pad region so later
                    # chunks' halo reads below PAD see window-correct bits
                    a = a0[k] if j == 0 else bs[j - 1]
                    nc.vector.tensor_tensor(
                        out=bufs[k + 1][:, a:b],
                        in0=bufs[k][:, a:b],
                        in1=bufs[k][:, a - s:b - s],
                        op=bor,
                    )
                olo = PAD if j == 0 else bs[j - 1]
                q = nc.sync if j % 2 == 0 else nc.scalar
                q.dma_start(out=y[:, olo - PAD:b - PAD], in_=bufs[nlev][:, olo:b])
    nc.finalize()
    return nc


def _get_built(d: int):
    if d not in _CACHE:
        _CACHE[d] = _build(d)
    return _CACHE[d]


def kernel(input_spikes, duration, _trace=False):
    from concourse.bass_utils import run_bass_kernel_spmd

    x = np.asarray(input_spikes)
    d = int(duration)
    assert x.shape == (T_FULL, B_FULL, N_FULL), x.shape

    if d <= 0:
        out = np.zeros(x.shape, dtype=np.float32)
        return (out, None) if _trace else out
    if d == 1:
        # window of length 1: output is the (binary) input itself
        out = (x != 0).astype(np.float32).reshape(x.shape)
        return (out, None) if _trace else out
    d = min(d, T_FULL)  # any window >= T covers [0, t] for every t

    PAD = _pad(d)
    nc = _get_built(d)

    bits = x.reshape(T_FULL, COLS) != 0
    in_maps = []
    for core in range(NCORES):
        # pack 32 spatial columns per uint32 (bit b of word w = col 32w+b),
        # transpose to [word, time], prepend PAD zero columns
        packed = np.packbits(
            bits[:, core * FREE_BITS:(core + 1) * FREE_BITS],
            axis=1, bitorder="little",
        ).view(np.uint32)
        arr = np.zeros((P, PAD + T_FULL), dtype=np.uint32)
        arr[:, PAD:] = packed.T
        in_maps.append({"x": arr})

    res = run_bass_kernel_spmd(
        nc, in_maps, core_ids=list(range(NCORES)), trace=_trace
    )

    out = np.empty((T_FULL, COLS), dtype=np.float32)
    for core in range(NCORES):
        yv = res.results[core]["y"]  # [P, T] uint32
        ub = np.unpackbits(
            np.ascontiguousarray(yv.T).view(np.uint8), axis=1, bitorder="little"
        )  # [T, FREE_BITS] uint8 in {0,1}
        out[:, core * FREE_BITS:(core + 1) * FREE_BITS] = ub
    out = out.reshape(T_FULL, B_FULL, N_FULL)
    if _trace:
        return out, res
    return out
